# revision 45
# baseline (speedup 1.0000x reference)
"""CapsRoute Trainium2 kernel: grouped conv -> capsule self-routing -> grouped conv.

Single software-pipelined loop over 5-row chunks: conv1 runs 5 chunks ahead,
routing (logits/softmax-gate/votes) in the middle, conv2 trails by 2-3 chunks
in pairs, so conv matmuls fill the PE gaps left by routing's cross-engine
dependencies.  Routing uses a (k, o-half) layout: E/ar tiles hold all 16 input
capsules x 8 output capsules per half, so every routing broadcast (th, r,
coeff-per-o) is a single matmul: 41 routing matmuls/chunk vs 64 in the naive
per-half layout.  The per-o coeff*pose products stay on DVE (PSUM-capable);
po4 vote matmuls write 32-row PE column-tile bands with zero-padded lhsT so no
PSUM pre-zeroing is needed.  Half the y2 scatter DMAs go through Pool's SWDGE
queue to keep the shared HWDGE below saturation.
"""
import numpy as np
import concourse.bass as bass
import concourse.bacc as bacc
import concourse.tile as tile
from concourse import mybir
from concourse._compat import with_exitstack
from contextlib import ExitStack

K_CAT, P_CAT, K_OUT, P_OUT = 16, 8, 16, 8
C_CAT = 144
EPS_ROUTE = 1e-6
BN_EPS = 1e-5
H = W = 96
HP = WP = 98
ROWS_PER_CHUNK = 5
CHUNKS = [(r, min(ROWS_PER_CHUNK, H - r)) for r in range(0, H, ROWS_PER_CHUNK)]
NCHUNK = len(CHUNKS)
N = ROWS_PER_CHUNK * W  # 480
PSN = 512
NPIX = H * W
POOL_CP_OS = set()  # all coeff-mults on DVE (Act/Pool detours regress)

F32 = mybir.dt.float32
BF16 = mybir.dt.bfloat16
AF = mybir.ActivationFunctionType
ALU = mybir.AluOpType


def prep_constants(conv_route_w, conv_route_gamma, conv_route_beta,
                   W_pose, W_gate, b_gate, spagg_w, spagg_gamma, spagg_beta):
    """Host-side constant prep. All lhsT arrays laid out [K_partition, free...]."""
    out = {}
    scale1 = (conv_route_gamma / np.sqrt(1.0 + BN_EPS)).astype(np.float32)
    scale2 = (spagg_gamma / np.sqrt(1.0 + BN_EPS)).astype(np.float32)

    # conv1 lhsT [72, 2, 9, 72]: [cin_local, half, tap, col j]
    # col j of conv1 psum: j<64 -> (k_loc=j//8, ch=j%8); j>=64 -> (k_loc=j-64, ch=8)
    c1 = np.zeros((72, 2, 9, 72), np.float32)
    w1 = conv_route_w * scale1[:, None, None, None]  # [144, 9, 3, 3]
    b1 = np.zeros((2, 72), np.float32)
    for h in range(2):
        for j in range(72):
            k_loc, ch = (j // 8, j % 8) if j < 64 else (j - 64, 8)
            cout = 72 * h + 9 * k_loc + ch
            for ci in range(9):
                for dy in range(3):
                    for dx in range(3):
                        c1[9 * k_loc + ci, h, 3 * dy + dx, j] = w1[cout, ci, dy, dx]
            b1[h, j] = conv_route_beta[cout]
    out["c1w"] = c1
    out["b1"] = b1

    # logits lhsT [128, 2, 128]: row 8k+p (pose layout), col 8k+o'
    wg2 = np.zeros((128, 2, 128), np.float32)
    bg2 = np.zeros((2, 128), np.float32)
    bg = b_gate.reshape(K_CAT, K_OUT)
    for half in range(2):
        for k in range(16):
            for op in range(8):
                bg2[half, 8 * k + op] = bg[k, 8 * half + op]
                for p in range(8):
                    wg2[8 * k + p, half, 8 * k + op] = W_gate[k, 8 * half + op, p]
    out["wg2"] = wg2
    out["bg2"] = bg2

    # S reduce: row 8k+o' -> col k
    ones_S2 = np.zeros((128, 16), np.float32)
    # th broadcast: row k -> col 8k+o'
    rtp_bc = np.zeros((16, 128), np.float32)
    # asum: row 8k+o' -> col (8*half + o')
    asum_l = np.zeros((128, 2, 16), np.float32)
    # r broadcast: row (8*half + o') -> col 8k+o'
    rrep_l = np.zeros((16, 2, 128), np.float32)
    # coeff broadcast per o': row 8k+o' -> cols 8k+p
    sel2 = np.zeros((128, 8, 128), np.float32)
    for k in range(16):
        for op in range(8):
            ones_S2[8 * k + op, k] = 1.0
            rtp_bc[k, 8 * k + op] = 1.0
            for half in range(2):
                asum_l[8 * k + op, half, 8 * half + op] = 1.0
                rrep_l[8 * half + op, half, 8 * k + op] = 1.0
            for p in range(8):
                sel2[8 * k + op, op, 8 * k + p] = 1.0
    out["ones_S2"] = ones_S2
    out["rtp_bc"] = rtp_bc
    out["asum_l"] = asum_l
    out["rrep_l"] = rrep_l
    out["sel2"] = sel2

    # wpose lhsT [128, 16, 32]: row 8k+p, [o], col q (cols 8:32 zero so each
    # po4 band matmul fully defines its 32-row PSUM band with start=True)
    wp = np.zeros((128, 16, 32), np.float32)
    for o in range(16):
        for k in range(16):
            for p in range(8):
                wp[8 * k + p, o, :8] = W_pose[k, o, p, :]
    out["wpose"] = wp

    # conv2 lhsT [72, 2, 9, 72] natural channel order
    c2 = np.zeros((72, 2, 9, 72), np.float32)
    w2 = spagg_w * scale2[:, None, None, None]
    for h in range(2):
        for j in range(72):
            cout = 72 * h + j
            g_loc = j // 9
            for ci in range(9):
                for dy in range(3):
                    for dx in range(3):
                        c2[9 * g_loc + ci, h, 3 * dy + dx, j] = w2[cout, ci, dy, dx]
    out["c2w"] = c2
    out["b2"] = spagg_beta.reshape(2, 72).astype(np.float32)
    for name, (shape, dt) in CONST_SPECS.items():
        want = mybir.dt.np(dt)
        out[name] = np.ascontiguousarray(out[name]).astype(want)
    return out


CONST_SPECS = {
    "c1w": ([72, 2, 9, 72], BF16),
    "b1": ([2, 72], F32),
    "wg2": ([128, 2, 128], BF16),
    "bg2": ([2, 128], F32),
    "ones_S2": ([128, 16], BF16),
    "rtp_bc": ([16, 128], BF16),
    "asum_l": ([128, 2, 16], BF16),
    "rrep_l": ([16, 2, 128], BF16),
    "sel2": ([128, 8, 128], BF16),
    "wpose": ([128, 16, 32], BF16),
    "c2w": ([72, 2, 9, 72], BF16),
    "b2": ([2, 72], F32),
}
BF16_NP = mybir.dt.np(BF16)


@with_exitstack
def capsroute_kernel(ctx: ExitStack, tc: tile.TileContext, outs, ins):
    nc = tc.nc
    out = outs["out"]

    singles = ctx.enter_context(tc.tile_pool(name="singles", bufs=1))
    xpool = ctx.enter_context(tc.tile_pool(name="xpool", bufs=1))
    y2pool = ctx.enter_context(tc.tile_pool(name="y2pool", bufs=1))
    rwork = ctx.enter_context(tc.tile_pool(name="rwork", bufs=3))

    cst = {}
    for name, (shape, dt) in CONST_SPECS.items():
        if name in ("b1", "b2", "bg2"):
            continue  # loaded as column vectors below
        t = singles.tile(shape, dt, name=f"{name}_c")
        nc.sync.dma_start(out=t[:], in_=ins[name][:])
        cst[name] = t

    bg_t = [singles.tile([128, 1], F32, name=f"bg{h}") for h in range(2)]
    b1_t = [singles.tile([72, 1], F32, name=f"b1_{h}") for h in range(2)]
    b2_t = [singles.tile([72, 1], F32, name=f"b2_{h}") for h in range(2)]
    for h in range(2):
        nc.sync.dma_start(out=bg_t[h][:], in_=ins["bg2"][h:h + 1, :].transpose([1, 0]))
        nc.sync.dma_start(out=b1_t[h][:], in_=ins["b1"][h:h + 1, :].transpose([1, 0]))
        nc.sync.dma_start(out=b2_t[h][:], in_=ins["b2"][h:h + 1, :].transpose([1, 0]))

    def win(t, r0, nr, dy, dx):
        rs = 1 + r0 + dy
        return t[:, rs:rs + nr, 1 + dx:1 + dx + W]

    def pad_border(t):
        nc.vector.memset(t[:, 0, :], 0.0)
        nc.vector.memset(t[:, 97, :], 0.0)
        nc.vector.memset(t[:, :, 0:1], 0.0)
        nc.vector.memset(t[:, :, 97:98], 0.0)

    xpad = [xpool.tile([72, HP, WP], BF16, name=f"xpad{h}") for h in range(2)]
    for h in range(2):
        pad_border(xpad[h])
    # interleave row-band loads of both inputs so warmup conv1 starts after
    # the first bands instead of waiting for the full 1.3MB loads
    XBANDS = [(0, 12), (12, 24), (24, 48), (48, 96)]
    for lo, hi in XBANDS:
        for h, xsrc in enumerate((ins["x0"], ins["x1"])):
            nc.gpsimd.dma_start(out=xpad[h][:, 1 + lo:1 + hi, 1:97],
                                in_=xsrc[:, lo:hi, :])

    y2 = [y2pool.tile([72, HP, WP], BF16, name=f"y2{h}") for h in range(2)]
    for h in range(2):
        pad_border(y2[h])

    pose_all = y2pool.tile([128, NPIX], BF16, name="pose_all")
    acty_all = y2pool.tile([16, NPIX], BF16, name="acty_all")
    as_img = y2pool.tile([16, NPIX], BF16, name="as_img")

    # ===== single software-pipelined loop: conv1 (lag 5) + routing +
    # ===== conv2 (lag 2/3 pairs); conv1 matmuls fill routing stalls
    C1_LAG = 1  # only needs to beat routing's pose/acty reads now
    with tc.tile_pool(name="c1psum", bufs=1, space="PSUM") as c1pool, \
         tc.tile_pool(name="Lps", bufs=1, space="PSUM") as Lpool, \
         tc.tile_pool(name="smallps", bufs=1, space="PSUM") as spool, \
         tc.tile_pool(name="repps", bufs=1, space="PSUM") as reppool, \
         tc.tile_pool(name="po4ps", bufs=1, space="PSUM") as po4pool:
        bcpool = spool  # S/asum/rtp/rr rotate one shared bank

        def conv1_half(c, h):
            r0, nr = CHUNKS[c]
            NC = nr * W
            sl = slice(r0 * W, r0 * W + NC)
            ps = Lpool.tile([128, PSN], F32, tag="L", name="c1ps", bufs=2)
            for tap in range(9):
                dy, dx = tap // 3 - 1, tap % 3 - 1
                nc.tensor.matmul(
                    ps[0:72, 0:NC], cst["c1w"][:, h, tap],
                    win(xpad[h], r0, nr, dy, dx),
                    start=(tap == 0), stop=(tap == 8))
            with nc.allow_low_precision(reason="bf16 pose"):
                if h == 0:
                    nc.scalar.activation(pose_all[0:64, sl], ps[0:64, 0:NC],
                                         AF.Silu, bias=b1_t[0][0:64])
                else:
                    ptmp = rwork.tile([64, N], BF16, tag="ptmp")
                    nc.scalar.activation(ptmp[:, 0:NC], ps[0:64, 0:NC],
                                         AF.Silu, bias=b1_t[1][0:64])
                    nc.sync.dma_start(out=pose_all[64:128, sl], in_=ptmp[:, 0:NC])
                acty_t = rwork.tile([72, N], BF16, tag=f"acty{h}")
                nc.scalar.activation(acty_t[64:72, 0:NC], ps[64:72, 0:NC],
                                     AF.Silu, bias=b1_t[h][64:72])
            eng = nc.scalar if h == 0 else nc.sync
            eng.dma_start(out=acty_all[8 * h:8 * h + 8, sl], in_=acty_t[64:72, 0:NC])

        def front_steps(c):
            """Generator: emits the per-chunk routing head in 5 stages."""
            r0, nr = CHUNKS[c]
            NC = nr * W
            sl = slice(r0 * W, r0 * W + NC)
            E = []
            for half in range(2):
                L = Lpool.tile([128, PSN], F32, tag="L", name="L", bufs=2)
                nc.tensor.matmul(L[:, 0:NC], cst["wg2"][:, half, :],
                                 pose_all[:, sl], start=True, stop=True)
                Eh = rwork.tile([128, N], BF16, tag=f"E{half}", bufs=4)
                nc.scalar.activation(Eh[:, 0:NC], L[:, 0:NC], AF.Exp,
                                     bias=bg_t[half][:])
                E.append(Eh)
            # sigmoid via the Exp table (no Sigmoid table load):
            # th = sigmoid(a)/S = 1/((1+exp(-a))*S)
            u = rwork.tile([16, N], BF16, tag="u")
            with nc.allow_low_precision(reason="bf16 routing"):
                nc.scalar.activation(u[:, 0:NC], acty_all[:, sl], AF.Exp, scale=-1.0)
            yield None
            S = spool.tile([128, PSN], F32, tag="small", name="S")
            for half in range(2):
                nc.tensor.matmul(S[0:16, 0:NC], cst["ones_S2"][:], E[half][:, 0:NC],
                                 start=(half == 0), stop=(half == 1))
            w = rwork.tile([16, N], F32, tag="rS")
            nc.vector.scalar_tensor_tensor(w[:, 0:NC], u[:, 0:NC], 1.0,
                                           S[0:16, 0:NC], op0=ALU.add, op1=ALU.mult)
            th = rwork.tile([16, N], BF16, tag="th")
            with nc.allow_low_precision(reason="bf16 routing"):
                nc.vector.reciprocal(th[:, 0:NC], w[:, 0:NC])
            yield None
            rtp = bcpool.tile([128, PSN], F32, tag="small", name="rtp")
            nc.tensor.matmul(rtp[:, 0:NC], cst["rtp_bc"][:], th[:, 0:NC],
                             start=True, stop=True)
            ar = []
            for half in range(2):
                arh = rwork.tile([128, N], BF16, tag=f"ar{half}", bufs=4)
                with nc.allow_low_precision(reason="bf16 routing"):
                    nc.vector.tensor_mul(arh[:, 0:NC], E[half][:, 0:NC], rtp[:, 0:NC])
                ar.append(arh)
            yield None
            asum = spool.tile([128, PSN], F32, tag="small", name="asum")
            for half in range(2):
                nc.tensor.matmul(asum[0:16, 0:NC], cst["asum_l"][:, half, :],
                                 ar[half][:, 0:NC], start=(half == 0), stop=(half == 1))
            with nc.allow_low_precision(reason="bf16 act img"):
                nc.vector.tensor_scalar_add(as_img[:, sl], asum[0:16, 0:NC], EPS_ROUTE)
                r = rwork.tile([16, N], BF16, tag="r")
                nc.vector.reciprocal(r[:, 0:NC], as_img[:, sl])
            yield None
            ar2 = []
            for half in range(2):
                rr = bcpool.tile([128, PSN], F32, tag="small", name="rr")
                nc.tensor.matmul(rr[:, 0:NC], cst["rrep_l"][:, half, :], r[:, 0:NC],
                                 start=True, stop=True)
                a2 = rwork.tile([128, N], BF16, tag=f"ar2{half}", bufs=4)
                with nc.allow_low_precision(reason="bf16 routing"):
                    nc.vector.tensor_mul(a2[:, 0:NC], ar[half][:, 0:NC], rr[:, 0:NC])
                ar2.append(a2)
            yield (c, ar2, {})

        def prep_pooled(state):
            """Early rep->Act-copy->Pool-mul for the Pool-assigned coeff mults,
            emitted an iteration ahead so their po4 consumers never stall."""
            c, ar2, cps = state
            r0, nr = CHUNKS[c]
            NC = nr * W
            sl = slice(r0 * W, r0 * W + NC)
            for o in sorted(POOL_CP_OS):
                rep = reppool.tile([128, PSN], F32, tag="rep", name="rep", bufs=2)
                nc.tensor.matmul(rep[:, 0:NC], cst["sel2"][:, o % 8, :],
                                 ar2[o // 8][:, 0:NC], start=True, stop=True)
                rep_sb = rwork.tile([128, N], BF16, tag="repsb", bufs=2)
                cp = rwork.tile([128, N], BF16, tag="cpp", bufs=7)
                with nc.allow_low_precision(reason="bf16 votes"):
                    nc.scalar.copy(rep_sb[:, 0:NC], rep[:, 0:NC])
                    nc.gpsimd.tensor_mul(cp[:, 0:NC], pose_all[:, sl],
                                         rep_sb[:, 0:NC])
                cps[o] = cp

        def emit_quad(state, quad):
            c, ar2, cps = state
            r0, nr = CHUNKS[c]
            NC = nr * W
            sl = slice(r0 * W, r0 * W + NC)
            po4 = po4pool.tile([128, PSN], F32, tag="po4", name="po4", bufs=2)
            for j in range(4):
                o = 4 * quad + j
                if o in cps:
                    continue
                rep = reppool.tile([128, PSN], F32, tag="rep", name="rep", bufs=2)
                nc.tensor.matmul(rep[:, 0:NC], cst["sel2"][:, o % 8, :],
                                 ar2[o // 8][:, 0:NC], start=True, stop=True)
                cp = rwork.tile([128, N], BF16, tag="cp", bufs=5)
                with nc.allow_low_precision(reason="bf16 votes"):
                    nc.vector.tensor_mul(cp[:, 0:NC], pose_all[:, sl],
                                         rep[:, 0:NC])
                cps[o] = cp
            for j in range(4):
                o = 4 * quad + j
                nc.tensor.matmul(po4[32 * j:32 * j + 32, 0:NC], cst["wpose"][:, o, :],
                                 cps[o][:, 0:NC], start=True, stop=True,
                                 skip_group_check=True, tile_position=(0, 32 * j))
            po4_sb = rwork.tile([128, N], BF16, tag="po4sb", bufs=3)
            with nc.allow_low_precision(reason="bf16 conv2 input"):
                nc.scalar.copy(po4_sb[:, 0:NC], po4[:, 0:NC])
            for j in range(4):
                o = 4 * quad + j
                h2, o_loc = o // 8, o % 8
                # odd j goes through Pool's SWDGE queue to offload HWDGE
                eng2 = (nc.sync if j == 0 else nc.scalar) if j % 2 == 0 else nc.gpsimd
                eng2.dma_start(
                    out=win(y2[h2], r0, nr, 0, 0)[9 * o_loc:9 * o_loc + 8],
                    in_=po4_sb[32 * j:32 * j + 8, 0:NC].rearrange(
                        "p (r w) -> p r w", w=W))

        def flush_act(c):
            r0, nr = CHUNKS[c]
            for h in range(2):
                eng = nc.scalar if h == 0 else nc.sync
                eng.dma_start(
                    out=y2[h][8:72:9, 1 + r0:1 + r0 + nr, 1:97],
                    in_=as_img[8 * h:8 * h + 8, r0 * W:r0 * W + nr * W].rearrange(
                        "p (r w) -> p r w", w=W))

        def conv2_chunk(c):
            r0, nr = CHUNKS[c]
            NC = nr * W
            for h in range(2):
                ps = po4pool.tile([72, PSN], F32, tag="c2ps", name="c2ps", bufs=1)
                for tap in range(9):
                    dy, dx = tap // 3 - 1, tap % 3 - 1
                    nc.tensor.matmul(
                        ps[:, 0:NC], cst["c2w"][:, h, tap],
                        win(y2[h], r0, nr, dy, dx),
                        start=(tap == 0), stop=(tap == 8))
                ob = rwork.tile([72, N], F32, tag="ob")
                nc.scalar.activation(ob[:, 0:NC], ps[:, 0:NC], AF.Silu,
                                     bias=b2_t[h][:])
                nc.sync.dma_start(
                    out=out[72 * h:72 * h + 72, r0:r0 + nr, :],
                    in_=ob[:, 0:NC].rearrange("p (r w) -> p r w", w=W))

        # warmup: conv1 for the first C1_LAG chunks + first sigmoid slab
        for wc in range(C1_LAG):
            conv1_half(wc, 0)
            conv1_half(wc, 1)

        prev = None
        next_c2 = 0
        for c in range(NCHUNK):
            gen = front_steps(c)
            next(gen)
            # trailing conv2 right after the logits stage: 18 dependency-free
            # matmuls of PE filler while DVE finishes the previous chunk's
            # coefficient tiles
            while next_c2 <= c - 3:
                conv2_chunk(next_c2)
                next_c2 += 1
            if prev is not None:
                prep_pooled(prev)
                emit_quad(prev, 0)
            if c + C1_LAG < NCHUNK:
                conv1_half(c + C1_LAG, 0)
            next(gen)
            if prev is not None:
                emit_quad(prev, 1)
            next(gen)
            if prev is not None:
                emit_quad(prev, 2)
            if c + C1_LAG < NCHUNK:
                conv1_half(c + C1_LAG, 1)
            next(gen)
            if prev is not None:
                emit_quad(prev, 3)
            cur = next(gen)
            if prev is not None:
                flush_act(prev[0])
            # near the end, start draining the conv2 tail at lag 2 (its quads
            # were just emitted) so less is left after the loop
            if c >= NCHUNK - 2:
                while next_c2 <= c - 2:
                    conv2_chunk(next_c2)
                    next_c2 += 1
            prev = cur
        prep_pooled(prev)
        for q in range(4):
            emit_quad(prev, q)
        flush_act(prev[0])
        while next_c2 < NCHUNK:
            conv2_chunk(next_c2)
            next_c2 += 1


def build_nc():
    nc = bacc.Bacc("TRN2", target_bir_lowering=False, debug=False)
    ins = {
        "x0": nc.dram_tensor("x0", [72, H, W], BF16, kind="ExternalInput").ap(),
        "x1": nc.dram_tensor("x1", [72, H, W], BF16, kind="ExternalInput").ap(),
    }
    for name, (shape, dt) in CONST_SPECS.items():
        ins[name] = nc.dram_tensor(name, shape, dt, kind="ExternalInput").ap()
    outs = {"out": nc.dram_tensor("out", [C_CAT, H, W], F32, kind="ExternalOutput").ap()}
    with tile.TileContext(nc) as tc:
        capsroute_kernel(tc, outs, ins)
    nc.compile()
    return nc

# ======================= host-side runner =======================
_NC_CACHE = {}


def _get_nc():
    if "nc" not in _NC_CACHE:
        _NC_CACHE["nc"] = build_nc()
    return _NC_CACHE["nc"]


def kernel(**inputs):
    """Full-batch entry point: shards batch 8 across 8 NeuronCores."""
    from concourse import bass_utils

    nc = _get_nc()
    consts = prep_constants(
        inputs["conv_route_w"].astype(np.float32),
        inputs["conv_route_gamma"].astype(np.float32),
        inputs["conv_route_beta"].astype(np.float32),
        inputs["W_pose"].astype(np.float32),
        inputs["W_gate"].astype(np.float32),
        inputs["b_gate"].astype(np.float32),
        inputs["spagg_w"].astype(np.float32),
        inputs["spagg_gamma"].astype(np.float32),
        inputs["spagg_beta"].astype(np.float32),
    )
    x0 = np.asarray(inputs["x0"]).astype(BF16_NP)
    x1 = np.asarray(inputs["x1"]).astype(BF16_NP)
    in_maps = []
    for b in range(8):
        m = dict(consts)
        m["x0"] = np.ascontiguousarray(x0[b])
        m["x1"] = np.ascontiguousarray(x1[b])
        in_maps.append(m)
    res = bass_utils.run_bass_kernel_spmd(nc, in_maps, core_ids=list(range(8)))
    out = np.stack([res.results[b]["out"] for b in range(8)], axis=0)
    return out.astype(np.float32)
